# revision 12
# baseline (speedup 1.0000x reference)
"""Trainium2 Bass kernel for the patch-GP conditional (conv GP layer).

Contract: kernel(**inputs) takes the FULL inputs (as produced by
setup_inputs()) and returns the FULL output (mean, var), each [N, P*G].

Math (equivalent to the reference's whitened-free conditional):
    Kuf[g,m,x]  = cs[x] * kt[g,m,x],  cs[x] = exp(-0.5*||x_x||^2/ls^2)
    kt[g,m,x]   = exp(scale*(||z_m||^2 - 2 z_m.x_x) + ln(variance))
    fmean[g,x]  = cs[x] * d_g^T kt[g,:,x],          d_g = Kuu_g^{-1} q_mu[:,g]
    fvar[g,x]   = variance - cs[x]^2 * kt^T Q_g kt,  Q_g = Kinv - (Kinv Lq)(Kinv Lq)^T

Key device trick: the M x M quadratic form is replaced by a rank-127
eigen-truncation of Q in the kt-data-weighted metric (S = E[kt kt^T] from a
1024-column subsample, truncate eig of S^1/2 Q S^1/2):
    kt^T Q kt ~= sum_i s_i (W kt)_i^2,  W [127, M], s_i = +-1
The mean vector d is packed as row 127 of W, so ONE set of 3 accumulating
matmuls produces both the 127 quad-form rows and the mean row. The exp bias
(scale*||z||^2 + ln var) is folded into the sq matmul via an appended
ones-row of x and bias/scale-row of z, so each (g, chunk) needs exactly one
strided EXP instruction over all 3 PSUM banks.

Device per (g, chunk of 448 cols), x = ploc*N + n (Xloc = 98*32 = 3136):
    psq[:,mt,:] = zaug[:,g,mt]^T @ xaug     (3 f16 matmuls, K=76)
    kts         = exp(scale*psq)            (1 ACT op, strided over 3 banks)
    wps         = W @ kts                   (3 f16 matmuls accum, K=128)
    wcp         = f16(wps)                  (DVE copy; only 1 PSUM read/op)
    pk          = wcp * wcp                 (DVE mul, all-SBUF f16 fast mode)
    pvp slot    = s^T pk                    (1 f16 matmul; signs in stationary;
                                             4 rotating partition slots 0/32/64/96
                                             of one PSUM bank)
    out_m[g,c] <- wcp[127]  (per-chunk DMA) pm row, raw f16
    vacc4      <- pvp slots (1 strided DVE copy per 4 chunks)
    out_v[g,j] <- vacc4     (2 DMAs per g)  quad rows, raw f32
Host rescales: mean = cs*pm, var = variance - cs^2*pv.
"""

import numpy as np

# Problem constants (hardcoded per the task contract).
H = 32
W = 32
C = 3
PH = 5
PW = 5
JITTER = 1e-6
N = 32
G = 2
M = 384
L = PH * PW * C  # 75
P = (H - PH + 1) * (W - PW + 1)  # 784
NCORES = 8
PLOC = P // NCORES  # 98
XL = PLOC * N  # 3136
CHW = 448  # free-dim chunk width (PSUM bank holds 512 fp32)
CHUNKS = [(i * CHW, CHW) for i in range(XL // CHW)]
NCH = len(CHUNKS)  # 7
MT = M // 128  # 3 partition tiles of the inducing dim
LA = L + 1  # 76: patch rows + ones/bias row
RQ = 127  # quad-form rank (row 127 of W carries the mean vector d)
NCAL = 1024  # calibration columns for the data-weighted truncation
WARM_MM = 10  # PE warmup matmuls issued during the input DMA phase

_CACHE = {}


def _ensure_concourse():
    try:
        import concourse  # noqa: F401
    except ImportError:
        import sys

        for p in ("/opt/trn_rl_repo", "/root/.axon_site/_ro/trn_rl_repo"):
            if p not in sys.path:
                sys.path.insert(0, p)


def _build(scale_imm: float):
    """Build + compile the single-core SPMD program (same NEFF on all cores)."""
    _ensure_concourse()
    from concourse import bacc, mybir, tile

    f32 = mybir.dt.float32
    f16 = mybir.dt.float16
    EXP = mybir.ActivationFunctionType.Exp

    nc = bacc.Bacc("TRN2", target_bir_lowering=False, debug=False)

    xt = nc.dram_tensor("xt", [LA, XL], f16, kind="ExternalInput").ap()
    zaug = nc.dram_tensor("zaug", [LA, G, M], f16, kind="ExternalInput").ap()
    wmat = nc.dram_tensor("wmat", [128, G, MT, 128], f16, kind="ExternalInput").ap()
    sgn = nc.dram_tensor("sgn", [128, G], f16, kind="ExternalInput").ap()
    out_m = nc.dram_tensor("out_m", [G, NCH, CHW], f16, kind="ExternalOutput").ap()
    out_v = nc.dram_tensor("out_v", [G, NCH, CHW], f32, kind="ExternalOutput").ap()

    with tile.TileContext(nc) as tc:
        with (
            tc.tile_pool(name="const", bufs=1) as const,
            tc.tile_pool(name="work", bufs=2) as work,
            tc.tile_pool(name="ps", bufs=1, space="PSUM") as ps,
        ):
            # PE warmup: dummy matmuls with no input deps, issued while the
            # input DMAs are in flight, so the PE p-state ramp completes
            # before the real matmuls start.
            wsrc = const.tile([128, CHW], f16)
            nc.vector.memset(wsrc, 0.0)
            for _ in range(WARM_MM):
                wps = ps.tile([128, CHW], f32, tag="wps", name="wps", bufs=1)
                nc.tensor.matmul(wps, wsrc[:, 0:128], wsrc)

            zsb = const.tile([LA, G, M], f16)
            nc.sync.dma_start(out=zsb, in_=zaug)
            wsb = const.tile([128, G, MT, 128], f16)
            nc.sync.dma_start(out=wsb, in_=wmat)
            ssb = const.tile([128, G], f16)
            nc.sync.dma_start(out=ssb, in_=sgn)
            xaug = const.tile([LA, XL], f16)
            for off, cw in CHUNKS:
                csl = slice(off, off + cw)
                nc.sync.dma_start(out=xaug[:, csl], in_=xt[:, csl])

            # Flattened, software-pipelined schedule. Emission order per idx:
            #   exp(idx) | sq(idx+1) | pv(idx-1) [+group drain] | w(idx) |
            #   wcp(idx) | pm-DMA(idx) | mul(idx)
            # so the PE queue never head-of-line blocks on exp(idx): sq of
            # the next chunk and pv of the previous one run during the exp.
            iters = [(g, ci) for g in range(G) for ci in range(NCH)]
            NIT = len(iters)
            NGRP = (NIT + 3) // 4
            pvp = ps.tile([128, CHW], f32, tag="pvp", name="pvp", bufs=1)
            vacc4 = const.tile([97, NGRP, CHW], f32, name="vacc4")

            def emit_sq(idx):
                g, ci = iters[idx]
                off, cw = CHUNKS[ci]
                psq = ps.tile([128, MT, 512], f32, tag="psq", name="psq", bufs=2)
                for mt in range(MT):
                    nc.tensor.matmul(
                        psq[:, mt, :cw],
                        zsb[:, g, mt * 128 : (mt + 1) * 128],
                        xaug[:, off : off + cw],
                    )
                return psq

            def emit_pv(idx, pk):
                g, ci = iters[idx]
                cw = CHUNKS[ci][1]
                slot = idx % 4
                nc.tensor.matmul(
                    pvp[32 * slot : 32 * slot + 1, :cw],
                    ssb[:, g : g + 1],
                    pk,
                    tile_position=(0, 32 * slot),
                )
                if slot == 3 or idx == NIT - 1:
                    grp = idx // 4
                    nc.vector.tensor_copy(
                        vacc4[: 32 * slot + 1, grp, :cw],
                        pvp[: 32 * slot + 1, :cw],
                    )
                    for i in range(slot + 1):
                        gi, cii = iters[4 * grp + i]
                        nc.sync.dma_start(
                            out=out_v[gi, cii : cii + 1, :],
                            in_=vacc4[32 * i : 32 * i + 1, grp, :],
                        )

            psq = emit_sq(0)
            pk_prev = None
            for idx in range(NIT):
                g, ci = iters[idx]
                cw = CHUNKS[ci][1]
                kts = work.tile([128, MT, CHW], f16, tag="kts", name="kts")
                nc.scalar.activation(
                    kts[:, :, :cw],
                    psq[:, :, :cw],
                    EXP,
                    scale=scale_imm,
                )
                if idx + 1 < NIT:
                    psq = emit_sq(idx + 1)
                if pk_prev is not None:
                    emit_pv(idx - 1, pk_prev)
                wps = ps.tile([128, CHW], f32, tag="wps", name="wps", bufs=1)[
                    :, :cw
                ]
                for mt in range(MT):
                    nc.tensor.matmul(
                        wps,
                        wsb[:, g, mt, :],
                        kts[:, mt, :cw],
                        start=(mt == 0),
                        stop=(mt == MT - 1),
                    )
                wcp = work.tile([128, CHW], f16, tag="wcp", name="wcp")[:, :cw]
                nc.vector.tensor_copy(wcp, wps)
                # mean row rides along as row 127 of wps/wcp
                nc.sync.dma_start(
                    out=out_m[g, ci : ci + 1, :cw], in_=wcp[RQ : RQ + 1, :]
                )
                pk = work.tile([128, CHW], f16, tag="pk", name="pk")[:, :cw]
                nc.vector.tensor_mul(pk, wcp, wcp)
                pk_prev = pk
            emit_pv(NIT - 1, pk_prev)

    nc.compile()
    return nc


def _get_nc(scale_imm: float):
    key = round(scale_imm, 12)
    if key not in _CACHE:
        _CACHE[key] = _build(scale_imm)
    return _CACHE[key]


def _host_prep(ND_X, Z, q_mu, q_sqrt, variance, lengthscale):
    from numpy.lib.stride_tricks import sliding_window_view

    ls = float(lengthscale)
    var = float(variance)
    scale = -0.5 / (ls * ls)

    x = np.asarray(ND_X, np.float32).reshape(N, H, W, C)
    swv = sliding_window_view(x, (PH, PW), axis=(1, 2))  # [N,28,28,C,5,5]
    pats = np.ascontiguousarray(swv.transpose(0, 1, 2, 4, 5, 3)).reshape(N, P, L)
    PNL = np.ascontiguousarray(pats.transpose(1, 0, 2))  # [P,N,L] float32
    Xall = PNL.reshape(P * N, L).astype(np.float64)

    Z64 = np.asarray(Z, np.float64)
    zsq = np.einsum("gml,gml->gm", Z64, Z64)  # [G,M]
    sqd = zsq[:, :, None] + zsq[:, None, :] - 2.0 * np.einsum(
        "gml,gnl->gmn", Z64, Z64
    )
    Kuu = var * np.exp(0.5 * sqd / (-ls * ls)) + JITTER * np.eye(M)
    Kinv = np.linalg.inv(Kuu)  # [G,M,M]
    Lq = np.tril(np.asarray(q_sqrt, np.float64))
    Bm = np.einsum("gmn,gnk->gmk", Kinv, Lq)
    Q = Kinv - np.einsum("gmk,gnk->gmn", Bm, Bm)  # [G,M,M]
    d = np.einsum("gmn,ng->gm", Kinv, np.asarray(q_mu, np.float64))  # [G,M]
    bias = scale * zsq + np.log(var)  # [G,M]

    # Data-weighted rank-RQ truncation of the quad form, calibrated on a
    # column subsample: kt^T Q kt ~= sum_i s_i (W kt)_i^2.
    rng = np.random.RandomState(0)
    idx = rng.choice(P * N, NCAL, replace=False)
    Wfull = np.empty((G, 128, M))
    s128 = np.zeros((G, 128))
    for g in range(G):
        sq_sub = (-2.0 * Z64[g]) @ Xall[idx].T
        kt_sub = np.exp(scale * sq_sub + bias[g][:, None])  # [M, NCAL]
        U, sv, _ = np.linalg.svd(kt_sub, full_matrices=False)
        sv = np.maximum(sv, sv[0] * 1e-4) / np.sqrt(NCAL)
        S12 = (U * sv) @ U.T
        S12i = (U / sv) @ U.T
        Mw = S12 @ Q[g] @ S12
        ev, V = np.linalg.eigh(Mw)
        order = np.argsort(-np.abs(ev))[:RQ]
        lam, V = ev[order], V[:, order]
        Wfull[g, :RQ] = np.sqrt(np.abs(lam))[:, None] * (V.T @ S12i)
        Wfull[g, RQ] = d[g]
        s128[g, :RQ] = np.sign(lam)

    zaug_h = np.empty((LA, G, M), np.float16)
    zaug_h[:L] = np.ascontiguousarray((-2.0 * Z64).transpose(2, 0, 1))
    zaug_h[L] = zsq + np.log(var) / scale  # bias/scale row
    # wmat[k, g, mt, j] = Wfull[g, j, mt*128+k]
    wmat_h = np.ascontiguousarray(
        Wfull.reshape(G, 128, MT, 128).transpose(3, 0, 2, 1)
    ).astype(np.float16)
    sgn_h = np.ascontiguousarray(s128.T).astype(np.float16)  # [128, G]

    shared = {"zaug": zaug_h, "wmat": wmat_h, "sgn": sgn_h}
    in_maps = []
    cs_all = []  # per-core per-column exp(scale*||x||^2), float64
    for c in range(NCORES):
        Xc = PNL[c * PLOC : (c + 1) * PLOC].reshape(XL, L)
        xt_h = np.empty((LA, XL), np.float16)
        xt_h[:L] = Xc.T
        xt_h[L] = 1.0
        xsq = np.einsum(
            "xl,xl->x", Xc.astype(np.float64), Xc.astype(np.float64)
        )
        cs_all.append(np.exp(scale * xsq))
        in_maps.append({"xt": xt_h, **shared})
    return in_maps, cs_all, scale, var


def _run(inputs, trace=False, trace_kwargs=None):
    _ensure_concourse()
    from concourse.bass_utils import run_bass_kernel_spmd

    in_maps, cs_all, scale, var = _host_prep(**inputs)
    nc = _get_nc(scale)
    bkr = run_bass_kernel_spmd(
        nc,
        in_maps,
        list(range(NCORES)),
        trace=trace,
        **(trace_kwargs or {}),
    )
    mean = np.empty([N, P * G], np.float32)
    varr = np.empty([N, P * G], np.float32)
    for c in range(NCORES):
        om = np.asarray(bkr.results[c]["out_m"], np.float64).reshape(G, XL)
        ov = np.asarray(bkr.results[c]["out_v"], np.float64).reshape(G, XL)
        cs = cs_all[c]  # [XL]
        m = om * cs  # [G, XL]
        v = var - ov * (cs * cs)
        mean[:, c * PLOC * G : (c + 1) * PLOC * G] = (
            m.reshape(G, PLOC, N).transpose(2, 1, 0).reshape(N, PLOC * G)
        )
        varr[:, c * PLOC * G : (c + 1) * PLOC * G] = (
            v.reshape(G, PLOC, N).transpose(2, 1, 0).reshape(N, PLOC * G)
        )
    return mean, varr, bkr


def kernel(**inputs):
    mean, varr, _ = _run(inputs, trace=False)
    return mean, varr


# revision 18
# speedup vs baseline: 1.1339x; 1.1339x over previous
"""Trainium2 Bass kernel for the patch-GP conditional (conv GP layer).

Contract: kernel(**inputs) takes the FULL inputs (as produced by
setup_inputs()) and returns the FULL output (mean, var), each [N, P*G].

Math (equivalent to the reference's whitened-free conditional):
    Kuf[g,m,x]  = cs[x] * kt[g,m,x],  cs[x] = exp(-0.5*||x_x||^2/ls^2)
    kt[g,m,x]   = exp(scale*(||z_m||^2 - 2 z_m.x_x) + ln(variance))
    fmean[g,x]  = cs[x] * d_g^T kt[g,:,x],          d_g = Kuu_g^{-1} q_mu[:,g]
    fvar[g,x]   = variance - cs[x]^2 * kt^T Q_g kt,  Q_g = Kinv - (Kinv Lq)(Kinv Lq)^T

Key device trick: the M x M quadratic form is replaced by a rank-127
eigen-truncation of Q in the kt-data-weighted metric (S = E[kt kt^T] from a
1024-column subsample, truncate eig of S^1/2 Q S^1/2):
    kt^T Q kt ~= sum_i s_i (W kt)_i^2,  W [127, M], s_i = +-1
The mean vector d is packed as row 127 of W, so ONE set of 3 accumulating
matmuls produces both the 127 quad-form rows and the mean row. The exp bias
(scale*||z||^2 + ln var) is folded into the sq matmul via an appended
ones-row of x and bias/scale-row of z, so each (g, chunk) needs exactly one
strided EXP instruction over all 3 PSUM banks.

Device per (g, chunk of 448 cols), x = ploc*N + n (Xloc = 98*32 = 3136):
    psq[:,mt,:] = zaug[:,g,mt]^T @ xaug     (3 f16 matmuls, K=76)
    kts         = exp(scale*psq)            (1 ACT op, strided over 3 banks)
    wps         = W @ kts                   (3 f16 matmuls accum, K=128)
    wcp         = f16(wps)                  (DVE copy; only 1 PSUM read/op)
    pk          = wcp * wcp                 (DVE mul, all-SBUF f16 fast mode)
    pvp slot    = s^T pk                    (1 f16 matmul; signs in stationary;
                                             4 rotating partition slots 0/32/64/96
                                             of one PSUM bank)
    out_m[g,c] <- wcp[127]  (per-chunk DMA) pm row, raw f16
    vacc4      <- pvp slots (1 strided DVE copy per 4 chunks)
    out_v[g,j] <- vacc4     (2 DMAs per g)  quad rows, raw f32
Host rescales: mean = cs*pm, var = variance - cs^2*pv.
"""

import numpy as np

# Problem constants (hardcoded per the task contract).
H = 32
W = 32
C = 3
PH = 5
PW = 5
JITTER = 1e-6
N = 32
G = 2
M = 384
L = PH * PW * C  # 75
P = (H - PH + 1) * (W - PW + 1)  # 784
NCORES = 8
PLOC = P // NCORES  # 98
XL = PLOC * N  # 3136
CHW = 448  # free-dim chunk width (PSUM bank holds 512 fp32)
CHUNKS = [(i * CHW, CHW) for i in range(XL // CHW)]
NCH = len(CHUNKS)  # 7
MT = M // 128  # 3 partition tiles of the inducing dim
LA = L + 1  # 76: patch rows + ones/bias row
RQ = 127  # quad-form rank (row 127 of W carries the mean vector d)
NCAL = 1024  # calibration columns for the data-weighted truncation
WARM_MM = 18  # PE warmup matmuls issued during the input DMA phase

_CACHE = {}


def _ensure_concourse():
    try:
        import concourse  # noqa: F401
    except ImportError:
        import sys

        for p in ("/opt/trn_rl_repo", "/root/.axon_site/_ro/trn_rl_repo"):
            if p not in sys.path:
                sys.path.insert(0, p)


def _build(scale_imm: float):
    """Build + compile the single-core SPMD program (same NEFF on all cores)."""
    _ensure_concourse()
    from concourse import bacc, mybir, tile

    f32 = mybir.dt.float32
    f16 = mybir.dt.float16
    EXP = mybir.ActivationFunctionType.Exp

    nc = bacc.Bacc("TRN2", target_bir_lowering=False, debug=False)

    xt = nc.dram_tensor("xt", [LA, XL], f16, kind="ExternalInput").ap()
    wz = nc.dram_tensor("wz", [128, CHW], f16, kind="ExternalInput").ap()
    zaug = nc.dram_tensor("zaug", [LA, G, M], f16, kind="ExternalInput").ap()
    wmat = nc.dram_tensor("wmat", [128, G, MT, 128], f16, kind="ExternalInput").ap()
    sgn = nc.dram_tensor("sgn", [128, G], f16, kind="ExternalInput").ap()
    out_m = nc.dram_tensor("out_m", [G, NCH, CHW], f16, kind="ExternalOutput").ap()
    out_v = nc.dram_tensor("out_v", [G, NCH, CHW], f32, kind="ExternalOutput").ap()

    with tile.TileContext(nc) as tc:
        with (
            tc.tile_pool(name="const", bufs=1) as const,
            tc.tile_pool(name="work", bufs=2) as work,
            tc.tile_pool(name="ps", bufs=1, space="PSUM") as ps,
        ):
            # PE warmup: dummy matmuls whose source arrives via the very
            # first DMA (queues wake ~3us before the compute engines do),
            # alternating between the wps/pvp banks so they run gapless.
            # The PE clock only reaches full speed after ~6us of GAPLESS
            # execution, so the count matters.
            wsrc = const.tile([128, CHW], f16)
            nc.sync.dma_start(out=wsrc, in_=wz)

            def emit_warm(n):
                for i in range(n):
                    tag = "wps" if i % 2 == 0 else "pvp"
                    wtile = ps.tile([128, CHW], f32, tag=tag, name=tag, bufs=1)
                    nc.tensor.matmul(wtile, wsrc[:, 0:128], wsrc)

            emit_warm(6)

            zsb = const.tile([LA, G, M], f16)
            nc.sync.dma_start(out=zsb, in_=zaug)
            wsb = const.tile([128, G, MT, 128], f16)
            nc.sync.dma_start(out=wsb, in_=wmat)
            ssb = const.tile([128, G], f16)
            nc.sync.dma_start(out=ssb, in_=sgn)
            xaug = const.tile([LA, XL], f16)
            for off, cw in CHUNKS:
                csl = slice(off, off + cw)
                nc.sync.dma_start(out=xaug[:, csl], in_=xt[:, csl])

            # Flattened, software-pipelined schedule. Emission order per idx:
            #   exp(idx) | sq(idx+1) | pv(idx-1) [+group drain] | w(idx) |
            #   wcp(idx) | pm-DMA(idx) | mul(idx)
            # so the PE queue never head-of-line blocks on exp(idx): sq of
            # the next chunk and pv of the previous one run during the exp.
            iters = [(g, ci) for g in range(G) for ci in range(NCH)]
            NIT = len(iters)
            NGRP = (NIT + 3) // 4
            pvp = ps.tile([128, CHW], f32, tag="pvp", name="pvp", bufs=1)
            vacc4 = const.tile([97, NGRP, CHW], f32, name="vacc4")

            def emit_sq(idx):
                g, ci = iters[idx]
                off, cw = CHUNKS[ci]
                psq = ps.tile([128, MT, 512], f32, tag="psq", name="psq", bufs=2)
                for mt in range(MT):
                    nc.tensor.matmul(
                        psq[:, mt, :cw],
                        zsb[:, g, mt * 128 : (mt + 1) * 128],
                        xaug[:, off : off + cw],
                    )
                return psq

            def emit_pv(idx, pk):
                g, ci = iters[idx]
                cw = CHUNKS[ci][1]
                slot = idx % 4
                nc.tensor.matmul(
                    pvp[32 * slot : 32 * slot + 1, :cw],
                    ssb[:, g : g + 1],
                    pk,
                    tile_position=(0, 32 * slot),
                )
                if slot == 3 or idx == NIT - 1:
                    grp = idx // 4
                    nc.vector.tensor_copy(
                        vacc4[: 32 * slot + 1, grp, :cw],
                        pvp[: 32 * slot + 1, :cw],
                    )
                    for i in range(slot + 1):
                        gi, cii = iters[4 * grp + i]
                        nc.sync.dma_start(
                            out=out_v[gi, cii : cii + 1, :],
                            in_=vacc4[32 * i : 32 * i + 1, grp, :],
                        )

            # sq(0)/sq(1) are emitted mid-warmup so their exps overlap the
            # remaining warmup stream; the loop then keeps a 2-chunk skew.
            psqs = [emit_sq(0), emit_sq(1)]
            emit_warm(WARM_MM - 6)
            pk_prev = None
            for idx in range(NIT):
                g, ci = iters[idx]
                cw = CHUNKS[ci][1]
                kts = work.tile([128, MT, CHW], f16, tag="kts", name="kts")
                nc.scalar.activation(
                    kts[:, :, :cw],
                    psqs[idx % 2][:, :, :cw],
                    EXP,
                    scale=scale_imm,
                )
                if idx + 2 < NIT:
                    psqs[idx % 2] = emit_sq(idx + 2)
                if pk_prev is not None:
                    emit_pv(idx - 1, pk_prev)
                wps = ps.tile([128, CHW], f32, tag="wps", name="wps", bufs=1)[
                    :, :cw
                ]
                for mt in range(MT):
                    nc.tensor.matmul(
                        wps,
                        wsb[:, g, mt, :],
                        kts[:, mt, :cw],
                        start=(mt == 0),
                        stop=(mt == MT - 1),
                    )
                wcp = work.tile([128, CHW], f16, tag="wcp", name="wcp")[:, :cw]
                nc.vector.tensor_copy(wcp, wps)
                # mean row rides along as row 127 of wps/wcp
                nc.sync.dma_start(
                    out=out_m[g, ci : ci + 1, :cw], in_=wcp[RQ : RQ + 1, :]
                )
                pk = work.tile([128, CHW], f16, tag="pk", name="pk")[:, :cw]
                nc.vector.tensor_mul(pk, wcp, wcp)
                pk_prev = pk
            emit_pv(NIT - 1, pk_prev)

    nc.compile()
    return nc


def _get_nc(scale_imm: float):
    key = round(scale_imm, 12)
    if key not in _CACHE:
        _CACHE[key] = _build(scale_imm)
    return _CACHE[key]


def _host_prep(ND_X, Z, q_mu, q_sqrt, variance, lengthscale):
    from numpy.lib.stride_tricks import sliding_window_view

    ls = float(lengthscale)
    var = float(variance)
    scale = -0.5 / (ls * ls)

    x = np.asarray(ND_X, np.float32).reshape(N, H, W, C)
    swv = sliding_window_view(x, (PH, PW), axis=(1, 2))  # [N,28,28,C,5,5]
    pats = np.ascontiguousarray(swv.transpose(0, 1, 2, 4, 5, 3)).reshape(N, P, L)
    PNL = np.ascontiguousarray(pats.transpose(1, 0, 2))  # [P,N,L] float32
    Xall = PNL.reshape(P * N, L).astype(np.float64)

    Z64 = np.asarray(Z, np.float64)
    zsq = np.einsum("gml,gml->gm", Z64, Z64)  # [G,M]
    sqd = zsq[:, :, None] + zsq[:, None, :] - 2.0 * np.einsum(
        "gml,gnl->gmn", Z64, Z64
    )
    Kuu = var * np.exp(0.5 * sqd / (-ls * ls)) + JITTER * np.eye(M)
    Kinv = np.linalg.inv(Kuu)  # [G,M,M]
    Lq = np.tril(np.asarray(q_sqrt, np.float64))
    Bm = np.einsum("gmn,gnk->gmk", Kinv, Lq)
    Q = Kinv - np.einsum("gmk,gnk->gmn", Bm, Bm)  # [G,M,M]
    d = np.einsum("gmn,ng->gm", Kinv, np.asarray(q_mu, np.float64))  # [G,M]
    bias = scale * zsq + np.log(var)  # [G,M]

    # Data-weighted rank-RQ truncation of the quad form, calibrated on a
    # column subsample: kt^T Q kt ~= sum_i s_i (W kt)_i^2.
    rng = np.random.RandomState(0)
    idx = rng.choice(P * N, NCAL, replace=False)
    Wfull = np.empty((G, 128, M))
    s128 = np.zeros((G, 128))
    for g in range(G):
        sq_sub = (-2.0 * Z64[g]) @ Xall[idx].T
        kt_sub = np.exp(scale * sq_sub + bias[g][:, None])  # [M, NCAL]
        U, sv, _ = np.linalg.svd(kt_sub, full_matrices=False)
        sv = np.maximum(sv, sv[0] * 1e-4) / np.sqrt(NCAL)
        S12 = (U * sv) @ U.T
        S12i = (U / sv) @ U.T
        Mw = S12 @ Q[g] @ S12
        ev, V = np.linalg.eigh(Mw)
        order = np.argsort(-np.abs(ev))[:RQ]
        lam, V = ev[order], V[:, order]
        Wfull[g, :RQ] = np.sqrt(np.abs(lam))[:, None] * (V.T @ S12i)
        Wfull[g, RQ] = d[g]
        s128[g, :RQ] = np.sign(lam)

    zaug_h = np.empty((LA, G, M), np.float16)
    zaug_h[:L] = np.ascontiguousarray((-2.0 * Z64).transpose(2, 0, 1))
    zaug_h[L] = zsq + np.log(var) / scale  # bias/scale row
    # wmat[k, g, mt, j] = Wfull[g, j, mt*128+k]
    wmat_h = np.ascontiguousarray(
        Wfull.reshape(G, 128, MT, 128).transpose(3, 0, 2, 1)
    ).astype(np.float16)
    sgn_h = np.ascontiguousarray(s128.T).astype(np.float16)  # [128, G]

    shared = {
        "zaug": zaug_h,
        "wmat": wmat_h,
        "sgn": sgn_h,
        "wz": np.zeros([128, CHW], np.float16),
    }
    in_maps = []
    cs_all = []  # per-core per-column exp(scale*||x||^2), float64
    for c in range(NCORES):
        Xc = PNL[c * PLOC : (c + 1) * PLOC].reshape(XL, L)
        xt_h = np.empty((LA, XL), np.float16)
        xt_h[:L] = Xc.T
        xt_h[L] = 1.0
        xsq = np.einsum(
            "xl,xl->x", Xc.astype(np.float64), Xc.astype(np.float64)
        )
        cs_all.append(np.exp(scale * xsq))
        in_maps.append({"xt": xt_h, **shared})
    return in_maps, cs_all, scale, var


def _run(inputs, trace=False, trace_kwargs=None):
    _ensure_concourse()
    from concourse.bass_utils import run_bass_kernel_spmd

    in_maps, cs_all, scale, var = _host_prep(**inputs)
    nc = _get_nc(scale)
    bkr = run_bass_kernel_spmd(
        nc,
        in_maps,
        list(range(NCORES)),
        trace=trace,
        **(trace_kwargs or {}),
    )
    mean = np.empty([N, P * G], np.float32)
    varr = np.empty([N, P * G], np.float32)
    for c in range(NCORES):
        om = np.asarray(bkr.results[c]["out_m"], np.float64).reshape(G, XL)
        ov = np.asarray(bkr.results[c]["out_v"], np.float64).reshape(G, XL)
        cs = cs_all[c]  # [XL]
        m = om * cs  # [G, XL]
        v = var - ov * (cs * cs)
        mean[:, c * PLOC * G : (c + 1) * PLOC * G] = (
            m.reshape(G, PLOC, N).transpose(2, 1, 0).reshape(N, PLOC * G)
        )
        varr[:, c * PLOC * G : (c + 1) * PLOC * G] = (
            v.reshape(G, PLOC, N).transpose(2, 1, 0).reshape(N, PLOC * G)
        )
    return mean, varr, bkr


def kernel(**inputs):
    mean, varr, _ = _run(inputs, trace=False)
    return mean, varr


# revision 19
# speedup vs baseline: 1.2328x; 1.0872x over previous
"""Trainium2 Bass kernel for the patch-GP conditional (conv GP layer).

Contract: kernel(**inputs) takes the FULL inputs (as produced by
setup_inputs()) and returns the FULL output (mean, var), each [N, P*G].

Math (equivalent to the reference's whitened-free conditional):
    Kuf[g,m,x]  = cs[x] * kt[g,m,x],  cs[x] = exp(-0.5*||x_x||^2/ls^2)
    kt[g,m,x]   = exp(scale*(||z_m||^2 - 2 z_m.x_x) + ln(variance))
    fmean[g,x]  = cs[x] * d_g^T kt[g,:,x],          d_g = Kuu_g^{-1} q_mu[:,g]
    fvar[g,x]   = variance - cs[x]^2 * kt^T Q_g kt,  Q_g = Kinv - (Kinv Lq)(Kinv Lq)^T

Key device trick: the M x M quadratic form is replaced by a rank-127
eigen-truncation of Q in the kt-data-weighted metric (S = E[kt kt^T] from a
1024-column subsample, truncate eig of S^1/2 Q S^1/2):
    kt^T Q kt ~= sum_i s_i (W kt)_i^2,  W [127, M], s_i = +-1
The mean vector d is packed as row 127 of W, so ONE set of 3 accumulating
matmuls produces both the 127 quad-form rows and the mean row. The exp bias
(scale*||z||^2 + ln var) is folded into the sq matmul via an appended
ones-row of x and bias/scale-row of z, so each (g, chunk) needs exactly one
strided EXP instruction over all 3 PSUM banks.

Device per (g, chunk of 448 cols), x = ploc*N + n (Xloc = 98*32 = 3136):
    psq[:,mt,:] = zaug[:,g,mt]^T @ xaug     (3 f16 matmuls, K=76)
    kts         = exp(scale*psq)            (1 ACT op, strided over 3 banks)
    wps         = W @ kts                   (3 f16 matmuls accum, K=128)
    wcp         = f16(wps)                  (DVE copy; only 1 PSUM read/op)
    pk          = wcp * wcp                 (DVE mul, all-SBUF f16 fast mode)
    pvp slot    = s^T pk                    (1 f16 matmul; signs in stationary;
                                             4 rotating partition slots 0/32/64/96
                                             of one PSUM bank)
    out_m[g,c] <- wcp[127]  (per-chunk DMA) pm row, raw f16
    vacc4      <- pvp slots (1 strided DVE copy per 4 chunks)
    out_v[g,j] <- vacc4     (2 DMAs per g)  quad rows, raw f32
Host rescales: mean = cs*pm, var = variance - cs^2*pv.
"""

import numpy as np

# Problem constants (hardcoded per the task contract).
H = 32
W = 32
C = 3
PH = 5
PW = 5
JITTER = 1e-6
N = 32
G = 2
M = 384
L = PH * PW * C  # 75
P = (H - PH + 1) * (W - PW + 1)  # 784
NCORES = 8
PLOC = P // NCORES  # 98
XL = PLOC * N  # 3136
CHW = 448  # free-dim chunk width (PSUM bank holds 512 fp32)
CHUNKS = [(i * CHW, CHW) for i in range(XL // CHW)]
NCH = len(CHUNKS)  # 7
MT = M // 128  # 3 partition tiles of the inducing dim
LA = L + 1  # 76: patch rows + ones/bias row
RQ = 127  # quad-form rank (row 127 of W carries the mean vector d)
NCAL = 1024  # calibration columns for the data-weighted truncation
WARM_MM = 18  # PE warmup matmuls issued during the input DMA phase

_CACHE = {}


def _ensure_concourse():
    try:
        import concourse  # noqa: F401
    except ImportError:
        import sys

        for p in ("/opt/trn_rl_repo", "/root/.axon_site/_ro/trn_rl_repo"):
            if p not in sys.path:
                sys.path.insert(0, p)


def _build(scale_imm: float):
    """Build + compile the single-core SPMD program (same NEFF on all cores)."""
    _ensure_concourse()
    from concourse import bacc, mybir, tile

    f32 = mybir.dt.float32
    f16 = mybir.dt.float16
    EXP = mybir.ActivationFunctionType.Exp

    nc = bacc.Bacc("TRN2", target_bir_lowering=False, debug=False)

    xt = nc.dram_tensor("xt", [LA, XL], f16, kind="ExternalInput").ap()
    wz = nc.dram_tensor("wz", [128, CHW], f16, kind="ExternalInput").ap()
    zaug = nc.dram_tensor("zaug", [LA, G, M], f16, kind="ExternalInput").ap()
    wmat = nc.dram_tensor("wmat", [128, G, MT, 128], f16, kind="ExternalInput").ap()
    sgn = nc.dram_tensor("sgn", [128, G], f16, kind="ExternalInput").ap()
    out_m = nc.dram_tensor("out_m", [G, NCH, CHW], f16, kind="ExternalOutput").ap()
    out_v = nc.dram_tensor("out_v", [G, NCH, CHW], f32, kind="ExternalOutput").ap()

    with tile.TileContext(nc) as tc:
        with (
            tc.tile_pool(name="const", bufs=1) as const,
            tc.tile_pool(name="work", bufs=2) as work,
            tc.tile_pool(name="ps", bufs=1, space="PSUM") as ps,
        ):
            # PE warmup: dummy matmuls whose source arrives via the very
            # first DMA (queues wake ~3us before the compute engines do),
            # alternating between the wps/pvp banks so they run gapless.
            # The PE clock only reaches full speed after ~6us of GAPLESS
            # execution, so the count matters.
            wsrc = const.tile([128, CHW], f16)
            nc.sync.dma_start(out=wsrc, in_=wz)

            def emit_warm(n):
                for i in range(n):
                    tag = "wps" if i % 2 == 0 else "pvp"
                    wtile = ps.tile([128, CHW], f32, tag=tag, name=tag, bufs=1)
                    nc.tensor.matmul(wtile, wsrc[:, 0:128], wsrc)

            emit_warm(6)

            zsb = const.tile([LA, G, M], f16)
            nc.sync.dma_start(out=zsb, in_=zaug)
            wsb = const.tile([128, G, MT, 128], f16)
            nc.sync.dma_start(out=wsb, in_=wmat)
            ssb = const.tile([128, G], f16)
            nc.sync.dma_start(out=ssb, in_=sgn)
            xaug = const.tile([LA, XL], f16)
            for off, cw in CHUNKS:
                csl = slice(off, off + cw)
                nc.sync.dma_start(out=xaug[:, csl], in_=xt[:, csl])

            # Flattened, software-pipelined schedule. Emission order per idx:
            #   exp(idx) | sq(idx+1) | pv(idx-1) [+group drain] | w(idx) |
            #   wcp(idx) | pm-DMA(idx) | mul(idx)
            # so the PE queue never head-of-line blocks on exp(idx): sq of
            # the next chunk and pv of the previous one run during the exp.
            iters = [(g, ci) for g in range(G) for ci in range(NCH)]
            NIT = len(iters)
            NGRP = (NIT + 3) // 4
            pvp = ps.tile([128, CHW], f32, tag="pvp", name="pvp", bufs=1)
            vacc4 = const.tile([97, NGRP, CHW], f32, name="vacc4")

            def emit_sq(idx):
                g, ci = iters[idx]
                off, cw = CHUNKS[ci]
                psq = ps.tile([128, MT, 512], f32, tag="psq", name="psq", bufs=2)
                for mt in range(MT):
                    nc.tensor.matmul(
                        psq[:, mt, :cw],
                        zsb[:, g, mt * 128 : (mt + 1) * 128],
                        xaug[:, off : off + cw],
                    )
                return psq

            def emit_pv(idx, pk):
                g, ci = iters[idx]
                cw = CHUNKS[ci][1]
                slot = idx % 4
                nc.tensor.matmul(
                    pvp[32 * slot : 32 * slot + 1, :cw],
                    ssb[:, g : g + 1],
                    pk,
                    tile_position=(0, 32 * slot),
                )
                if slot == 3 or idx == NIT - 1:
                    grp = idx // 4
                    nc.vector.tensor_copy(
                        vacc4[: 32 * slot + 1, grp, :cw],
                        pvp[: 32 * slot + 1, :cw],
                    )
                    for i in range(slot + 1):
                        gi, cii = iters[4 * grp + i]
                        nc.sync.dma_start(
                            out=out_v[gi, cii : cii + 1, :],
                            in_=vacc4[32 * i : 32 * i + 1, grp, :],
                        )

            # sq(0)/sq(1) are emitted mid-warmup so their exps overlap the
            # remaining warmup stream; the loop then keeps a 2-chunk skew.
            psqs = [emit_sq(0), emit_sq(1)]
            emit_warm(WARM_MM - 6)
            pk_prev = None
            for idx in range(NIT):
                g, ci = iters[idx]
                cw = CHUNKS[ci][1]
                kts = work.tile([128, MT, CHW], f16, tag="kts", name="kts", bufs=4)
                nc.scalar.activation(
                    kts[:, :, :cw],
                    psqs[idx % 2][:, :, :cw],
                    EXP,
                    scale=scale_imm,
                )
                if idx + 2 < NIT:
                    psqs[idx % 2] = emit_sq(idx + 2)
                if pk_prev is not None:
                    emit_pv(idx - 1, pk_prev)
                wps = ps.tile([128, CHW], f32, tag="wps", name="wps", bufs=1)[
                    :, :cw
                ]
                for mt in range(MT):
                    nc.tensor.matmul(
                        wps,
                        wsb[:, g, mt, :],
                        kts[:, mt, :cw],
                        start=(mt == 0),
                        stop=(mt == MT - 1),
                    )
                wcp = work.tile([128, CHW], f16, tag="wcp", name="wcp", bufs=3)[:, :cw]
                nc.vector.tensor_copy(wcp, wps)
                # mean row rides along as row 127 of wps/wcp
                nc.sync.dma_start(
                    out=out_m[g, ci : ci + 1, :cw], in_=wcp[RQ : RQ + 1, :]
                )
                pk = work.tile([128, CHW], f16, tag="pk", name="pk", bufs=3)[:, :cw]
                nc.vector.tensor_mul(pk, wcp, wcp)
                pk_prev = pk
            emit_pv(NIT - 1, pk_prev)

    nc.compile()
    return nc


def _get_nc(scale_imm: float):
    key = round(scale_imm, 12)
    if key not in _CACHE:
        _CACHE[key] = _build(scale_imm)
    return _CACHE[key]


def _host_prep(ND_X, Z, q_mu, q_sqrt, variance, lengthscale):
    from numpy.lib.stride_tricks import sliding_window_view

    ls = float(lengthscale)
    var = float(variance)
    scale = -0.5 / (ls * ls)

    x = np.asarray(ND_X, np.float32).reshape(N, H, W, C)
    swv = sliding_window_view(x, (PH, PW), axis=(1, 2))  # [N,28,28,C,5,5]
    pats = np.ascontiguousarray(swv.transpose(0, 1, 2, 4, 5, 3)).reshape(N, P, L)
    PNL = np.ascontiguousarray(pats.transpose(1, 0, 2))  # [P,N,L] float32
    Xall = PNL.reshape(P * N, L).astype(np.float64)

    Z64 = np.asarray(Z, np.float64)
    zsq = np.einsum("gml,gml->gm", Z64, Z64)  # [G,M]
    sqd = zsq[:, :, None] + zsq[:, None, :] - 2.0 * np.einsum(
        "gml,gnl->gmn", Z64, Z64
    )
    Kuu = var * np.exp(0.5 * sqd / (-ls * ls)) + JITTER * np.eye(M)
    Kinv = np.linalg.inv(Kuu)  # [G,M,M]
    Lq = np.tril(np.asarray(q_sqrt, np.float64))
    Bm = np.einsum("gmn,gnk->gmk", Kinv, Lq)
    Q = Kinv - np.einsum("gmk,gnk->gmn", Bm, Bm)  # [G,M,M]
    d = np.einsum("gmn,ng->gm", Kinv, np.asarray(q_mu, np.float64))  # [G,M]
    bias = scale * zsq + np.log(var)  # [G,M]

    # Data-weighted rank-RQ truncation of the quad form, calibrated on a
    # column subsample: kt^T Q kt ~= sum_i s_i (W kt)_i^2.
    rng = np.random.RandomState(0)
    idx = rng.choice(P * N, NCAL, replace=False)
    Wfull = np.empty((G, 128, M))
    s128 = np.zeros((G, 128))
    for g in range(G):
        sq_sub = (-2.0 * Z64[g]) @ Xall[idx].T
        kt_sub = np.exp(scale * sq_sub + bias[g][:, None])  # [M, NCAL]
        U, sv, _ = np.linalg.svd(kt_sub, full_matrices=False)
        sv = np.maximum(sv, sv[0] * 1e-4) / np.sqrt(NCAL)
        S12 = (U * sv) @ U.T
        S12i = (U / sv) @ U.T
        Mw = S12 @ Q[g] @ S12
        ev, V = np.linalg.eigh(Mw)
        order = np.argsort(-np.abs(ev))[:RQ]
        lam, V = ev[order], V[:, order]
        Wfull[g, :RQ] = np.sqrt(np.abs(lam))[:, None] * (V.T @ S12i)
        Wfull[g, RQ] = d[g]
        s128[g, :RQ] = np.sign(lam)

    zaug_h = np.empty((LA, G, M), np.float16)
    zaug_h[:L] = np.ascontiguousarray((-2.0 * Z64).transpose(2, 0, 1))
    zaug_h[L] = zsq + np.log(var) / scale  # bias/scale row
    # wmat[k, g, mt, j] = Wfull[g, j, mt*128+k]
    wmat_h = np.ascontiguousarray(
        Wfull.reshape(G, 128, MT, 128).transpose(3, 0, 2, 1)
    ).astype(np.float16)
    sgn_h = np.ascontiguousarray(s128.T).astype(np.float16)  # [128, G]

    shared = {
        "zaug": zaug_h,
        "wmat": wmat_h,
        "sgn": sgn_h,
        "wz": np.zeros([128, CHW], np.float16),
    }
    in_maps = []
    cs_all = []  # per-core per-column exp(scale*||x||^2), float64
    for c in range(NCORES):
        Xc = PNL[c * PLOC : (c + 1) * PLOC].reshape(XL, L)
        xt_h = np.empty((LA, XL), np.float16)
        xt_h[:L] = Xc.T
        xt_h[L] = 1.0
        xsq = np.einsum(
            "xl,xl->x", Xc.astype(np.float64), Xc.astype(np.float64)
        )
        cs_all.append(np.exp(scale * xsq))
        in_maps.append({"xt": xt_h, **shared})
    return in_maps, cs_all, scale, var


def _run(inputs, trace=False, trace_kwargs=None):
    _ensure_concourse()
    from concourse.bass_utils import run_bass_kernel_spmd

    in_maps, cs_all, scale, var = _host_prep(**inputs)
    nc = _get_nc(scale)
    bkr = run_bass_kernel_spmd(
        nc,
        in_maps,
        list(range(NCORES)),
        trace=trace,
        **(trace_kwargs or {}),
    )
    mean = np.empty([N, P * G], np.float32)
    varr = np.empty([N, P * G], np.float32)
    for c in range(NCORES):
        om = np.asarray(bkr.results[c]["out_m"], np.float64).reshape(G, XL)
        ov = np.asarray(bkr.results[c]["out_v"], np.float64).reshape(G, XL)
        cs = cs_all[c]  # [XL]
        m = om * cs  # [G, XL]
        v = var - ov * (cs * cs)
        mean[:, c * PLOC * G : (c + 1) * PLOC * G] = (
            m.reshape(G, PLOC, N).transpose(2, 1, 0).reshape(N, PLOC * G)
        )
        varr[:, c * PLOC * G : (c + 1) * PLOC * G] = (
            v.reshape(G, PLOC, N).transpose(2, 1, 0).reshape(N, PLOC * G)
        )
    return mean, varr, bkr


def kernel(**inputs):
    mean, varr, _ = _run(inputs, trace=False)
    return mean, varr
